# revision 7
# baseline (speedup 1.0000x reference)
"""Bilinear affine image sampling on 8 Trainium2 cores (data parallel over N).

The axon tunnel to the NeuronCores moves ~43 MB/s aggregate (shared cap;
parallel streams/duplex don't scale it), so wall time == wire bytes.
Strategy: quantize I to int8 on host with per-(image,channel,row) scales
(25 MB up instead of 100 MB; ~0.7% l2 error), compute in f32 on device,
return int8 with per-output-row scales computed in the Bass kernel
(25 MB down; ~0.75% more, total 1.09e-2 vs the 2e-2 gate). The f32 row
scales are bit-packed into a 16-byte tail of the int8 output tensor so
one np.asarray fetches everything (~0.2s fixed cost per asarray call).

Per image n (core k owns images 4k..4k+3):
  1. host: per-(c,row) abs-max scale, int8 quantize, upload       [async]
  2. XLA prep jit: dequantize, sample coords hx,hy; x0/wx as in the
     reference's clamp-to-border semantics, CHW->HWC transpose    [device]
  3. XLA gather jit: ONE lax.gather with slice_sizes (2,2,3) fetches all
     four bilinear corners for every output pixel. Kept as a standalone
     jit whose operands are module parameters: that lowers to the 9-BIR-
     instruction runtime-DGE gather; fusing it with producers makes the
     tensorizer unroll 65536 static DMA instructions and crash walrus.
  4. Bass/Tile kernel (concourse.bass2jax.bass_jit — the same bass_exec
     custom-call path run_bass_kernel_spmd uses under axon) does the full
     bilinear blend for the core's 4 images on the vector engine, takes
     per-output-row abs-max, and emits round-to-nearest int8 plus the
     packed f32 row scales                                        [device]

The 8 per-core int8 outputs are viewed as one global sharded jax array,
materialized with a single np.asarray (25 MB on the wire), then
dequantized to f32 on host in one fused numpy pass.

(Per-element gather inside raw Bass is not viable on this runtime: gpsimd
ap_gather ~162ns/index, Pool INDIRECT_COPY crashes, and the toolchain
disables vector_dynamic_offsets DGE; the gather therefore runs as an XLA
op on the NeuronCores while the blend arithmetic runs in the Bass kernel.)
"""

import sys
import numpy as np

sys.path.insert(0, "/opt/trn_rl_repo")

N, C, H, W = 32, 3, 512, 512
NCORES = 8
IPC = N // NCORES        # images per core
HW = H * W
F = HW // 128            # 2048 free elems per partition
FC = 512                 # blend chunk along F

_cache = {}


def _build():
    import jax
    import jax.numpy as jnp
    from jax import lax
    from jax.sharding import Mesh, PartitionSpec, NamedSharding
    from concourse.bass2jax import bass_jit
    import concourse.mybir as mybir
    from concourse import tile

    devs = jax.devices()[:NCORES]
    mesh = Mesh(np.asarray(devs), ("core",))
    gsharding = NamedSharding(mesh, PartitionSpec("core"))

    @jax.jit
    def prep(q, s, a, t):
        # q [3,512,512] int8, s [3,512] f32 (rowmax/127), a [2,2], t [2]
        f32 = jnp.float32
        img = q.astype(f32) * s[:, :, None]
        cx = f32((H - 1) / 2.0)
        cy = f32((W - 1) / 2.0)
        xi = (jnp.arange(H, dtype=f32) - cx)[:, None]
        yj = (jnp.arange(W, dtype=f32) - cy)[None, :]
        hx = a[0, 0] * xi + a[0, 1] * yj + t[0] + cx
        hy = a[1, 0] * xi + a[1, 1] * yj + t[1] + cy
        x0 = jnp.clip(jnp.floor(hx), 0.0, f32(H - 2))
        y0 = jnp.clip(jnp.floor(hy), 0.0, f32(W - 2))
        wx = jnp.clip(hx - x0, 0.0, 1.0)
        wy = jnp.clip(hy - y0, 0.0, 1.0)
        starts = jnp.stack(
            [x0.astype(jnp.int32).reshape(HW), y0.astype(jnp.int32).reshape(HW)],
            axis=1)
        hwc = jnp.transpose(img, (1, 2, 0))
        return hwc, starts, wx.reshape(128, F), wy.reshape(128, F)

    dn = lax.GatherDimensionNumbers(
        offset_dims=(1, 2, 3), collapsed_slice_dims=(), start_index_map=(0, 1))

    @jax.jit
    def corners(hwc, starts):
        g = lax.gather(hwc, starts, dn, slice_sizes=(2, 2, C),
                       mode=lax.GatherScatterMode.PROMISE_IN_BOUNDS)
        return g.reshape(128, F, 2 * 2 * C)

    sub = mybir.AluOpType.subtract
    add = mybir.AluOpType.add
    mult = mybir.AluOpType.mult

    MAGIC = 12582912.0  # 1.5*2^23: (x+MAGIC)-MAGIC == round-to-nearest(x)

    @bass_jit(trn_type="TRN2")
    def blend(nc, g0, g1, g2, g3, wx0, wx1, wx2, wx3, wy0, wy1, wy2, wy3):
        # g* [128,F,12] f32, wx*/wy* [128,F] f32
        # -> out int8 [IPC,C,128,F] + per-output-row abs-max sc [IPC,C,128,F/FC]
        # (FC=512 means each (partition, chunk) holds exactly one output row)
        gs = [g0, g1, g2, g3]
        wxs = [wx0, wx1, wx2, wx3]
        wys = [wy0, wy1, wy2, wy3]
        # data in [:, :, :, :F]; the per-row f32 scales bit-packed into the
        # 16-byte tail [F:F+16] so ONE np.asarray fetches everything
        out_d = nc.dram_tensor(
            "out", [IPC, C, 128, F + 16], mybir.dt.int8, kind="ExternalOutput")
        with tile.TileContext(nc) as tc:
            with tc.tile_pool(name="sbuf", bufs=2) as pool:
                for m in range(IPC):
                    rm4s = [pool.tile([128, F // FC], mybir.dt.float32,
                                      name=f"rm4_{m}_{c}", tag=f"rm4{c}")
                            for c in range(C)]
                    for fc in range(F // FC):
                        sl = slice(fc * FC, (fc + 1) * FC)
                        gt = pool.tile([128, FC, 4 * C], mybir.dt.float32, tag="g")
                        wxt = pool.tile([128, FC], mybir.dt.float32, tag="wx")
                        wyt = pool.tile([128, FC], mybir.dt.float32, tag="wy")
                        nc.sync.dma_start(gt[:, :, :], gs[m][:, sl, :])
                        nc.sync.dma_start(wxt[:], wxs[m][:, sl])
                        nc.sync.dma_start(wyt[:], wys[m][:, sl])
                        for c in range(C):
                            t01 = pool.tile([128, FC], mybir.dt.float32, tag="t01")
                            t11 = pool.tile([128, FC], mybir.dt.float32, tag="t11")
                            rmax = rm4s[c][:, fc:fc + 1]
                            rinv = pool.tile([128, 1], mybir.dt.float32, tag="rinv")
                            ob = pool.tile([128, FC], mybir.dt.int8, tag=f"ob{c}")
                            p00 = gt[:, :, 0 * C + c]
                            p01 = gt[:, :, 1 * C + c]
                            p10 = gt[:, :, 2 * C + c]
                            p11 = gt[:, :, 3 * C + c]
                            # top = p00 + wy*(p01-p00)
                            nc.vector.tensor_tensor(t01[:], p01, p00, sub)
                            nc.vector.tensor_tensor(t01[:], t01[:], wyt[:], mult)
                            nc.vector.tensor_tensor(t01[:], t01[:], p00, add)
                            # bot = p10 + wy*(p11-p10)
                            nc.vector.tensor_tensor(t11[:], p11, p10, sub)
                            nc.vector.tensor_tensor(t11[:], t11[:], wyt[:], mult)
                            nc.vector.tensor_tensor(t11[:], t11[:], p10, add)
                            # out = top + wx*(bot-top)
                            nc.vector.tensor_tensor(t11[:], t11[:], t01[:], sub)
                            nc.vector.tensor_tensor(t11[:], t11[:], wxt[:], mult)
                            nc.vector.tensor_tensor(t11[:], t11[:], t01[:], add)
                            # per-row (per-partition) abs-max -> int8 quantize
                            nc.vector.tensor_reduce(
                                rmax, t11[:], axis=mybir.AxisListType.X,
                                op=mybir.AluOpType.max, apply_absolute_value=True)
                            nc.vector.tensor_scalar_max(rmax, rmax, 1e-30)
                            nc.vector.reciprocal(rinv[:], rmax)
                            # t01 := round(t11 * (127/rmax))
                            nc.vector.tensor_scalar(
                                t01[:], t11[:], rinv[:], 127.0, op0=mult, op1=mult)
                            nc.vector.tensor_scalar_add(t01[:], t01[:], MAGIC)
                            nc.vector.tensor_scalar_sub(t01[:], t01[:], MAGIC)
                            nc.vector.tensor_copy(ob[:], t01[:])
                            nc.sync.dma_start(out_d[m, c, :, sl], ob[:])
                    for c in range(C):
                        nc.sync.dma_start(
                            out_d[m, c, :, F:F + 16].bitcast(mybir.dt.float32),
                            rm4s[c][:, :])
        return out_d

    _cache.update(
        jax=jax, devs=devs, gsharding=gsharding,
        prep=prep, corners=corners, blend=blend,
        mk=jax.make_array_from_single_device_arrays,
    )


def kernel(I, A, T):
    if not _cache:
        _build()
    jax = _cache["jax"]
    devs = _cache["devs"]
    prep = _cache["prep"]
    corners = _cache["corners"]
    blend = _cache["blend"]

    I = np.asarray(I, dtype=np.float32)
    A = np.asarray(A, dtype=np.float32)
    T = np.asarray(T, dtype=np.float32)

    # quantize + upload + dispatch, image-major so all cores start early;
    # everything below is async until np.asarray
    gq = [[None] * IPC for _ in range(NCORES)]
    wxs = [[None] * IPC for _ in range(NCORES)]
    wys = [[None] * IPC for _ in range(NCORES)]
    for m in range(IPC):
        for k in range(NCORES):
            n = k * IPC + m
            blk = I[n]                                   # [3,512,512]
            rm = np.abs(blk).max(axis=2)                 # [3,512]
            np.maximum(rm, 1e-30, out=rm)
            q = np.rint(blk * (127.0 / rm)[:, :, None]).astype(np.int8)
            qd = jax.device_put(q, devs[k])
            sd = jax.device_put(rm / 127.0, devs[k])
            ad = jax.device_put(A[n], devs[k])
            td = jax.device_put(T[n], devs[k])
            hwc, starts, wx, wy = prep(qd, sd, ad, td)
            gq[k][m] = corners(hwc, starts)
            wxs[k][m] = wx
            wys[k][m] = wy

    packed = [blend(*gq[k], *wxs[k], *wys[k]) for k in range(NCORES)]

    # ONE global fetch (np.asarray has ~0.2s fixed cost per call, so both
    # per-core streamed fetches and a separate scales fetch are losses);
    # prime the D2H early so the fetch overlaps the compute tail
    mk = _cache["mk"]
    gsh = _cache["gsharding"]
    garr = mk((N, C, 128, F + 16), gsh, packed)
    try:
        garr.copy_to_host_async()
    except Exception:
        pass
    res = np.asarray(garr)                               # int8 [N,C,128,F+16]
    sc = np.ascontiguousarray(res[..., F:]).view(np.float32)  # [N,C,128,F/FC]
    # one fused pass: int8 -> f32 cast and scale multiply together
    vs = res[..., :F].reshape(N, C, 128, F // FC, FC)    # view (last-axis split)
    out = np.multiply(vs, (sc * (1.0 / 127.0))[..., None], dtype=np.float32)
    return out.reshape(N, C, H, W)
